# revision 1
# baseline (speedup 1.0000x reference)
"""Banded diagonal gather (sparse local attention window) on 8 trn2 cores.

out[b, i, j] = x[b, i, i + j] if i + j < L else 0,  for j in [0, 256).

Key layout fact: in the row-major flat batch x[b], the band for row i starts
at flat offset i * (L + 1).  Declaring the per-core input DRAM tensor with
shape [ROWS, L + 1] therefore turns the diagonal gather into plain
rectangular slices: band rows == x2d[:, 0:LIMIT].

Sharding: 8 shards = batch(4) x sequence-half(2). Core c = b*2 + h handles
rows [h*2048, (h+1)*2048) of batch b. Fully independent, no collectives.

Per-core program:
  - rows 0..1791 (always fully in-band): one DRAM->DRAM strided DMA.
  - rows 1792..2047: load to SBUF, multiply by a 0/1 mask (all-ones for
    first-half cores, lower-trapezoid for second-half cores where the band
    runs past the sequence end), store. Masked lanes read in-bounds garbage
    (zero-padded tail), multiply-by-zero makes them exact 0.0f.
"""

import sys

for _p in ("/opt/trn_rl_repo",):
    if _p not in sys.path:
        sys.path.insert(0, _p)

import numpy as np

import concourse.bass as bass
import concourse.mybir as mybir
from concourse.bass_utils import run_bass_kernel_spmd

B = 4
L = 4096
LIMIT = 256
ROWS = 2048          # rows per core
PITCH = L + 1        # 4097
CLEAN = ROWS - 2 * 128   # 1792 rows handled by the big DMA
N_CORES = 8

_F32 = mybir.dt.float32


def _build_program() -> bass.Bass:
    nc = bass.Bass()
    x = nc.dram_tensor("x", [ROWS, PITCH], _F32, kind="ExternalInput")
    m = nc.dram_tensor("mask", [128, 512], _F32, kind="ExternalInput")
    out = nc.dram_tensor("out", [ROWS, LIMIT], _F32, kind="ExternalOutput")

    with (
        nc.sbuf_tensor([128, 512], _F32) as tail,
        nc.sbuf_tensor([128, 512], _F32) as msk,
        nc.semaphore("bsem") as bsem,
        nc.semaphore("lsem") as lsem,
        nc.semaphore("ssem") as ssem,
        nc.semaphore("vsem") as vsem,
        nc.Block() as block,
    ):

        @block.scalar
        def _(scalar):
            # Bulk of the work: strided band read -> contiguous band write.
            scalar.dma_start(
                out=out[0:CLEAN, :], in_=x[0:CLEAN, 0:LIMIT]
            ).then_inc(bsem, 16)
            scalar.wait_ge(bsem, 16)

        @block.sync
        def _(sync):
            sync.dma_start(out=msk[:, :], in_=m[:, :]).then_inc(lsem, 16)
            sync.dma_start(
                out=tail[:, 0:256], in_=x[CLEAN : CLEAN + 128, 0:LIMIT]
            ).then_inc(lsem, 16)
            sync.dma_start(
                out=tail[:, 256:512], in_=x[CLEAN + 128 : ROWS, 0:LIMIT]
            ).then_inc(lsem, 16)
            sync.wait_ge(vsem, 1)
            sync.dma_start(
                out=out[CLEAN : CLEAN + 128, :], in_=tail[:, 0:256]
            ).then_inc(ssem, 16)
            sync.dma_start(
                out=out[CLEAN + 128 : ROWS, :], in_=tail[:, 256:512]
            ).then_inc(ssem, 16)
            sync.wait_ge(ssem, 32)

        @block.vector
        def _(vector):
            vector.wait_ge(lsem, 48)
            vector.tensor_mul(tail[:, :], tail[:, :], msk[:, :]).then_inc(vsem, 1)

    return nc


def _build_in_maps(x: np.ndarray) -> list[dict[str, np.ndarray]]:
    xc = np.ascontiguousarray(np.asarray(x, dtype=np.float32))
    n = ROWS * PITCH  # 8_390_656; also == start offset of the second half

    ones_mask = np.ones((128, 512), dtype=np.float32)
    r = np.arange(128, dtype=np.int64)[:, None]
    j = np.arange(256, dtype=np.int64)[None, :]
    tri_mask = np.concatenate(
        [(j < 256 - r), (j < 128 - r)], axis=1
    ).astype(np.float32)

    in_maps = []
    for b in range(B):
        flat = xc[b].reshape(-1)
        # h=0: band starts at 0, fits entirely -> zero-copy view.
        h0 = flat[:n].reshape(ROWS, PITCH)
        # h=1: band starts at 2048*4097 == n; pad the overhang with zeros.
        buf = np.zeros(n, dtype=np.float32)
        avail = flat.size - n
        buf[:avail] = flat[n:]
        h1 = buf.reshape(ROWS, PITCH)
        in_maps.append({"x": h0, "mask": ones_mask})
        in_maps.append({"x": h1, "mask": tri_mask})
    return in_maps


_NC_CACHE = None


def kernel(x: np.ndarray) -> np.ndarray:
    global _NC_CACHE
    if _NC_CACHE is None:
        _NC_CACHE = _build_program()
    in_maps = _build_in_maps(x)
    res = run_bass_kernel_spmd(_NC_CACHE, in_maps, list(range(N_CORES))).results
    out = np.empty((B, L, LIMIT), dtype=np.float32)
    for c in range(N_CORES):
        b, h = divmod(c, 2)
        out[b, h * ROWS : (h + 1) * ROWS, :] = res[c]["out"]
    return out


# revision 4
# speedup vs baseline: 1.1344x; 1.1344x over previous
"""Banded diagonal gather (sparse local attention window) on 8 trn2 cores.

out[b, i, j] = x[b, i, i + j] if i + j < L else 0,  for j in [0, 256).

Key layout fact: in the row-major flat batch x[b], the band for row i starts
at flat offset i * (L + 1).  Declaring the per-core input DRAM tensor with
shape [ROWS, L + 1] therefore turns the diagonal gather into plain
rectangular slices: the banded output is exactly x2d[:, 0:LIMIT], and the
device program is a pure strided DMA copy (per core: 2 MiB HBM read +
2 MiB HBM write - the memory floor for this op).

Sharding: 8 shards = batch(4) x sequence-half(2). Core c = b*2 + h handles
rows [h*2048, (h+1)*2048) of batch b. Fully independent, no collectives.

Masking: row bands are DISJOINT intervals of the flat buffer (stride 4097 >
width 256), so a band position past the sequence end is read by no other
row. Second-half cores need a host-built padded copy anyway (their window
overruns the batch); the invalid triangle positions are zeroed in that
copy, so the device program needs no masking at all.

The copy is split into 4 chunks alternated across the two HWDGE rings
(sync=SP, scalar=ACT) so both descriptor generators and DMA queues run in
parallel. Each engine clears its own completion semaphore before use
(race-free: only that engine's DMAs increment it), making the kernel
robust to stale semaphore state left by previously crashed NEFFs.
"""

import sys

for _p in ("/opt/trn_rl_repo",):
    if _p not in sys.path:
        sys.path.insert(0, _p)

import numpy as np

import concourse.bass as bass
import concourse.mybir as mybir
from concourse.bass_utils import run_bass_kernel_spmd

B = 4
L = 4096
LIMIT = 256
ROWS = 2048          # rows per core
PITCH = L + 1        # 4097
N_CORES = 8
N_CHUNKS = 4

_F32 = mybir.dt.float32


def _build_program() -> bass.Bass:
    nc = bass.Bass()
    x = nc.dram_tensor("x", [ROWS, PITCH], _F32, kind="ExternalInput")
    out = nc.dram_tensor("out", [ROWS, LIMIT], _F32, kind="ExternalOutput")

    rows_per = ROWS // N_CHUNKS
    chunks = [(i * rows_per, (i + 1) * rows_per) for i in range(N_CHUNKS)]
    sync_chunks = chunks[0::2]
    scalar_chunks = chunks[1::2]

    with (
        nc.semaphore("ssem") as ssem,
        nc.semaphore("asem") as asem,
        nc.Block() as block,
    ):

        @block.sync
        def _(sync):
            sync.sem_clear(ssem)
            for lo, hi in sync_chunks:
                sync.dma_start(out=out[lo:hi, :], in_=x[lo:hi, 0:LIMIT]).then_inc(
                    ssem, 16
                )
            sync.wait_ge(ssem, 16 * len(sync_chunks))

        @block.scalar
        def _(scalar):
            scalar.sem_clear(asem)
            for lo, hi in scalar_chunks:
                scalar.dma_start(
                    out=out[lo:hi, :], in_=x[lo:hi, 0:LIMIT]
                ).then_inc(asem, 16)
            scalar.wait_ge(asem, 16 * len(scalar_chunks))

    return nc


def _build_in_maps(x: np.ndarray) -> list[dict[str, np.ndarray]]:
    xc = np.ascontiguousarray(np.asarray(x, dtype=np.float32))
    n = ROWS * PITCH  # 8_390_656; also == flat start offset of the 2nd half

    in_maps = []
    for b in range(B):
        flat = xc[b].reshape(-1)
        # h=0: band starts at offset 0 and fits entirely; every row is fully
        # in-band (max col = 2047+255 < 4096) -> zero-copy strided view.
        h0 = flat[:n].reshape(ROWS, PITCH)
        # h=1: band starts at flat offset n; pad the overhang with zeros and
        # zero the invalid triangle (row p keeps 2048-p valid elements for
        # p > 1792; bands are disjoint intervals so this clobbers nothing).
        buf = np.zeros(n, dtype=np.float32)
        avail = flat.size - n
        buf[:avail] = flat[n:]
        for p in range(ROWS - LIMIT + 1, ROWS):
            valid = ROWS - p
            buf[p * PITCH + valid : p * PITCH + LIMIT] = 0.0
        h1 = buf.reshape(ROWS, PITCH)
        in_maps.append({"x": h0})
        in_maps.append({"x": h1})
    return in_maps


_NC_CACHE = None


def kernel(x: np.ndarray) -> np.ndarray:
    global _NC_CACHE
    if _NC_CACHE is None:
        _NC_CACHE = _build_program()
    in_maps = _build_in_maps(x)
    res = run_bass_kernel_spmd(_NC_CACHE, in_maps, list(range(N_CORES))).results
    out = np.empty((B, L, LIMIT), dtype=np.float32)
    for c in range(N_CORES):
        b, h = divmod(c, 2)
        out[b, h * ROWS : (h + 1) * ROWS, :] = res[c]["out"]
    return out


# revision 5
# speedup vs baseline: 1.1611x; 1.0236x over previous
"""Banded diagonal gather (sparse local attention window) on 8 trn2 cores.

out[b, i, j] = x[b, i, i + j] if i + j < L else 0,  for j in [0, 256).

Key layout fact: in the row-major flat batch x[b], the band for row i starts
at flat offset i * (L + 1).  Declaring the per-core input DRAM tensor with
shape [ROWS, L + 1] therefore turns the diagonal gather into plain
rectangular slices: the banded output is exactly x2d[:, 0:LIMIT], and the
device program is a pure strided DMA copy (per core: 2 MiB HBM read +
2 MiB HBM write - the memory floor for this op).

Sharding: 8 shards = batch(4) x sequence-half(2). Core c = b*2 + h handles
rows [h*2048, (h+1)*2048) of batch b. Fully independent, no collectives.

Masking: row bands are DISJOINT intervals of the flat buffer (stride 4097 >
width 256), so a band position past the sequence end is read by no other
row. Second-half cores need a host-built padded copy anyway (their window
overruns the batch); the invalid triangle positions are zeroed in that
copy, so the device program needs no masking at all.

The copy is split into 4 chunks alternated across the two HWDGE rings
(sync=SP, scalar=ACT) so both descriptor generators and DMA queues run in
parallel. Each engine clears its own completion semaphore before use
(race-free: only that engine's DMAs increment it), making the kernel
robust to stale semaphore state left by previously crashed NEFFs.
"""

import sys

for _p in ("/opt/trn_rl_repo",):
    if _p not in sys.path:
        sys.path.insert(0, _p)

import numpy as np

import concourse.bass as bass
import concourse.mybir as mybir
from concourse.bass_utils import run_bass_kernel_spmd

B = 4
L = 4096
LIMIT = 256
ROWS = 2048          # rows per core
PITCH = L + 1        # 4097
N_CORES = 8
N_CHUNKS = 4

_F32 = mybir.dt.float32


def _build_program(n_chunks: int = N_CHUNKS) -> bass.Bass:
    nc = bass.Bass()
    x = nc.dram_tensor("x", [ROWS, PITCH], _F32, kind="ExternalInput")
    out = nc.dram_tensor("out", [ROWS, LIMIT], _F32, kind="ExternalOutput")

    rows_per = ROWS // n_chunks
    chunks = [(i * rows_per, (i + 1) * rows_per) for i in range(n_chunks)]
    sync_chunks = chunks[0::2]
    scalar_chunks = chunks[1::2]

    with (
        nc.semaphore("ssem") as ssem,
        nc.semaphore("asem") as asem,
        nc.Block() as block,
    ):

        @block.sync
        def _(sync):
            sync.sem_clear(ssem)
            for lo, hi in sync_chunks:
                sync.dma_start(out=out[lo:hi, :], in_=x[lo:hi, 0:LIMIT]).then_inc(
                    ssem, 16
                )
            sync.wait_ge(ssem, 16 * len(sync_chunks))

        @block.scalar
        def _(scalar):
            scalar.sem_clear(asem)
            for lo, hi in scalar_chunks:
                scalar.dma_start(
                    out=out[lo:hi, :], in_=x[lo:hi, 0:LIMIT]
                ).then_inc(asem, 16)
            scalar.wait_ge(asem, 16 * len(scalar_chunks))

    return nc


def _build_in_maps(x: np.ndarray) -> list[dict[str, np.ndarray]]:
    xc = np.ascontiguousarray(np.asarray(x, dtype=np.float32))
    n = ROWS * PITCH  # 8_390_656; also == flat start offset of the 2nd half

    in_maps = []
    for b in range(B):
        flat = xc[b].reshape(-1)
        # h=0: band starts at offset 0 and fits entirely; every row is fully
        # in-band (max col = 2047+255 < 4096) -> zero-copy strided view.
        h0 = flat[:n].reshape(ROWS, PITCH)
        # h=1: band starts at flat offset n; pad the overhang with zeros and
        # zero the invalid triangle (row p keeps 2048-p valid elements for
        # p > 1792; bands are disjoint intervals so this clobbers nothing).
        buf = np.zeros(n, dtype=np.float32)
        avail = flat.size - n
        buf[:avail] = flat[n:]
        for p in range(ROWS - LIMIT + 1, ROWS):
            valid = ROWS - p
            buf[p * PITCH + valid : p * PITCH + LIMIT] = 0.0
        h1 = buf.reshape(ROWS, PITCH)
        in_maps.append({"x": h0})
        in_maps.append({"x": h1})
    return in_maps


_NC_CACHE = None


def kernel(x: np.ndarray) -> np.ndarray:
    global _NC_CACHE
    if _NC_CACHE is None:
        _NC_CACHE = _build_program()
    in_maps = _build_in_maps(x)
    res = run_bass_kernel_spmd(_NC_CACHE, in_maps, list(range(N_CORES))).results
    out = np.empty((B, L, LIMIT), dtype=np.float32)
    for c in range(N_CORES):
        b, h = divmod(c, 2)
        out[b, h * ROWS : (h + 1) * ROWS, :] = res[c]["out"]
    return out
